# revision 9
# baseline (speedup 1.0000x reference)
"""Trainium2 Bass kernel for a 3-layer stacked LSTM (T=32768, batch=1) + linear head.

Strategy
--------
The recurrence is strictly sequential, but the LSTM here is strongly
contracting (weights scaled by 0.1 => forget gates ~0.5), so:

1. Split the sequence into 8 chunks, one per NeuronCore, each with a short
   zero-state warmup prefix (W=64) whose influence decays below fp32 noise.
   Core 0 starts at t=0 (its zero init is exact); cores 1..7 discard the
   first W outputs.

2. On each core, solve each LSTM layer over its whole window by fixed-point
   (Jacobi) iteration instead of stepping: guess the h-trajectory (zeros),
   compute ALL gate pre-activations with large batched matmuls
   (W_ih @ x + W_hh @ h_guess + b), run the cell-state recurrence
   c_t = f_t*c_{t-1} + i_t*g_t for the whole window with chunked
   tensor_tensor_scan instructions, recompute h = o*tanh(c), and repeat.
   The iteration contracts ~0.3-0.5x per pass; 12-16 passes per layer hit
   ~1e-5 relative error.  Everything is big, dense, engine-friendly work.

Most matmuls run as float32r (full PE rate); the last FINAL_FP32 iterations
of each layer use exact fp32 matmuls, which pins the fixed point (and hence
the final error) at fp32-matmul accuracy while paying the 4x fp32 PE cost
only on those final passes.  Tensors feeding matmuls are *declared*
float32r (same bit layout as fp32 — rounding happens inside the PE) so the
BIR verifier's fp32r producer/consumer dtype rule is satisfied; the exact
iterations read the same bits bitcast back to fp32.
"""

import numpy as np

import concourse.bass as bass
import concourse.tile as tile
from concourse import mybir

# Problem shape (hardcoded; the harness provides matching inputs).
T = 32768
IN, H1, H2, H3 = 8, 51, 100, 100
N_CORES = 8
WARM = 64
CH = T // N_CORES          # 4096 output steps per core
TC = CH + WARM             # 4160 processed steps per core
SEC = 512                  # matmul moving-dim section (one PSUM bank)
CHUNK = 1024               # act/psum chunk (2 PSUM banks)
ITERS = (10, 13, 13)       # Jacobi iterations per layer
FINAL_FP32 = 0             # last n iterations per layer use fp32 matmuls
                           # (measured: fp32 and fp32r matmuls are numerically
                           #  identical on this hardware, so exact passes buy
                           #  nothing -- keep 0)
PSUM_BUFS = 4
WORK_BUFS = 2
H_BUFS = 2
IG_ENG = "vector"          # engine for ig = i*g
HM_ENG = "vector"          # engine for h = o*tanh(c)
SCAN_ENG = "vector"        # engine for the cell-state scan

_SIG = mybir.ActivationFunctionType.Sigmoid
_TANH = mybir.ActivationFunctionType.Tanh
_IDENT = mybir.ActivationFunctionType.Identity
_F32 = mybir.dt.float32
_F32R = mybir.dt.float32r


_MAX_WAITS = 1  # this walrus build allows one sync-wait command per instruction


def _split_multi_waits(nc):
    """Move all but one sync wait off each instruction onto same-engine NOPs
    inserted directly before it (engines execute block-order sequentially, so
    waiting on a preceding NOP is equivalent)."""
    n = 0
    for f in nc.m.functions:
        for bb in f.blocks:
            out = []
            for inst in bb.instructions:
                si = inst.sync_info
                if si is not None and len(si.on_wait) > _MAX_WAITS:
                    waits = list(si.on_wait)
                    for w in waits[: -_MAX_WAITS]:
                        nop = mybir.InstNoOp(
                            name=f"splitwait_nop_{n}", engine=inst.engine
                        )
                        n += 1
                        nop.sync_info = mybir.SyncInfo(on_wait=[w], on_update=[])
                        out.append(nop)
                    si.on_wait = waits[-_MAX_WAITS:]
                out.append(inst)
            bb.instructions[:] = out


def _chunks(tc, chunk):
    return [(a, min(a + chunk, tc)) for a in range(0, tc, chunk)]


def _mm(nc, out_ap, lhsT, rhs, start, stop, exact):
    """lhsT/rhs are float32r APs; bitcast to fp32 for exact iterations."""
    if exact:
        lhsT = lhsT.bitcast(_F32)
        rhs = rhs.bitcast(_F32)
    nc.tensor.matmul(out_ap, lhsT, rhs, start=start, stop=stop)


def _run_layer(
    nc,
    pools,
    tc_len,
    kin,
    h,
    in_ap,          # [kin, tc_len] SBUF AP (float32r) of the layer input
    wihT,           # [kin, 4h] SBUF tile (float32r)
    whhT,           # [h, 4h] SBUF tile (float32r)
    bias,           # [h, 4] SBUF tile (fp32)
    n_iters,
    n_final_fp32,
    traj_tile,      # [h, 1+tc_len] float32r tile for the final h trajectory
):
    """Jacobi-iterate one LSTM layer; returns the last iteration's c tile.

    All work inside an iteration is chunk-local (CHUNK wide): the four gate
    matmul groups land in per-gate PSUM tiles, activations pull them into
    chunk-sized SBUF tiles, and the ig/scan/tanh/h chain follows per chunk.
    Only c (scan chaining + final c3) and h (next iteration's matmul input)
    are full-length."""
    work, hpool, psum = pools
    gate_funcs = (_SIG, _SIG, _TANH, _SIG)  # i, f, g, o
    chunks = _chunks(tc_len, CHUNK)
    secs = _chunks(tc_len, SEC)
    h_prev = None
    c_sb = None
    for k in range(n_iters):
        exact = k >= n_iters - n_final_fp32
        last = k == n_iters - 1
        h_dst = traj_tile if last else hpool.tile([h, 1 + tc_len], _F32R, tag="h")
        nc.gpsimd.memset(h_dst[:, 0:1].bitcast(_F32), 0.0)
        c_sb = work.tile([h, tc_len], _F32, tag="c")
        for c0, c1 in chunks:
            cl = c1 - c0
            gates = []
            for gi in range(4):
                wih_g = wihT[:, gi * h : (gi + 1) * h]
                whh_g = whhT[:, gi * h : (gi + 1) * h]
                ps = psum.tile([128, CHUNK], _F32, tag="ps")
                for s0, s1 in secs:
                    if s1 <= c0 or s0 >= c1:
                        continue
                    o_ap = ps[0:h, s0 - c0 : s1 - c0]
                    _mm(nc, o_ap, wih_g, in_ap[:, s0:s1], True, k == 0, exact)
                    if k > 0:
                        _mm(nc, o_ap, whh_g, h_prev[:, s0:s1], False, True, exact)
                gt = work.tile([h, CHUNK], _F32, tag=f"g{gi}")
                nc.scalar.activation(
                    gt[:, 0:cl],
                    ps[0:h, 0:cl],
                    gate_funcs[gi],
                    bias=bias[:, gi : gi + 1],
                )
                gates.append(gt)
            i_sb, f_sb, g_sb, o_sb = gates
            getattr(nc, IG_ENG).tensor_mul(g_sb[:, 0:cl], i_sb[:, 0:cl], g_sb[:, 0:cl])
            init = 0.0 if c0 == 0 else c_sb[:, c0 - 1 : c0]
            getattr(nc, SCAN_ENG).tensor_tensor_scan(
                c_sb[:, c0:c1],
                f_sb[:, 0:cl],
                g_sb[:, 0:cl],
                init,
                mybir.AluOpType.mult,
                mybir.AluOpType.add,
            )
            nc.scalar.activation(f_sb[:, 0:cl], c_sb[:, c0:c1], _TANH)
            getattr(nc, HM_ENG).tensor_mul(
                h_dst[:, 1 + c0 : 1 + c1], o_sb[:, 0:cl], f_sb[:, 0:cl]
            )
        h_prev = h_dst
    return c_sb


def _build_program():
    nc = bass.Bass(
        "TRN2", target_bir_lowering=False, debug=False, num_devices=N_CORES
    )
    d = {}
    def inp(name, shape, dt=_F32R):
        d[name] = nc.dram_tensor(name, list(shape), dt, kind="ExternalInput").ap()
    inp("xT", (IN, TC))
    inp("wih1T", (IN, 4 * H1)); inp("whh1T", (H1, 4 * H1)); inp("b1", (H1, 4), _F32)
    inp("wih2T", (H1, 4 * H2)); inp("whh2T", (H2, 4 * H2)); inp("b2", (H2, 4), _F32)
    inp("wih3T", (H2, 4 * H3)); inp("whh3T", (H3, 4 * H3)); inp("b3", (H3, 4), _F32)
    inp("wlinT", (H3, 1), _F32); inp("blin", (1, 1), _F32)
    outs_d = nc.dram_tensor("outs", [1, TC], _F32, kind="ExternalOutput").ap()
    h3_d = nc.dram_tensor("h3_last", [H3, 1], _F32R, kind="ExternalOutput").ap()
    c3_d = nc.dram_tensor("c3_last", [H3, 1], _F32, kind="ExternalOutput").ap()

    with tile.TileContext(nc) as tc:
        with (
            tc.tile_pool(name="wts", bufs=1) as wts,
            tc.tile_pool(name="xin", bufs=1) as xin,
            tc.tile_pool(name="work", bufs=WORK_BUFS) as work,
            tc.tile_pool(name="hiter", bufs=H_BUFS) as hpool,
            tc.tile_pool(name="traj", bufs=2) as traj,
            tc.tile_pool(name="olin", bufs=2) as olin,
            tc.tile_pool(name="psum", bufs=PSUM_BUFS, space="PSUM") as psum,
        ):
            wt = {}
            for name, shape, dt in [
                ("wih1T", (IN, 4 * H1), _F32R), ("whh1T", (H1, 4 * H1), _F32R),
                ("b1", (H1, 4), _F32),
                ("wih2T", (H1, 4 * H2), _F32R), ("whh2T", (H2, 4 * H2), _F32R),
                ("b2", (H2, 4), _F32),
                ("wih3T", (H2, 4 * H3), _F32R), ("whh3T", (H3, 4 * H3), _F32R),
                ("b3", (H3, 4), _F32),
                ("wlinT", (H3, 1), _F32), ("blin", (1, 1), _F32),
            ]:
                t = wts.tile(list(shape), dt, tag=name)
                nc.sync.dma_start(t[:, :], d[name][:, :])
                wt[name] = t
            x_t = xin.tile([IN, TC], _F32R)
            nc.sync.dma_start(x_t[:, :], d["xT"][:, :])

            pools = (work, hpool, psum)
            h1_traj = traj.tile([H1, 1 + TC], _F32R, tag="traj")
            _run_layer(nc, pools, TC, IN, H1, x_t[:, :], wt["wih1T"], wt["whh1T"],
                       wt["b1"], ITERS[0], FINAL_FP32, h1_traj)
            h2_traj = traj.tile([H2, 1 + TC], _F32R, tag="traj")
            _run_layer(nc, pools, TC, H1, H2, h1_traj[:, 1:], wt["wih2T"],
                       wt["whh2T"], wt["b2"], ITERS[1], FINAL_FP32, h2_traj)
            h3_traj = traj.tile([H3, 1 + TC], _F32R, tag="traj")
            c3_sb = _run_layer(nc, pools, TC, H2, H3, h2_traj[:, 1:], wt["wih3T"],
                               wt["whh3T"], wt["b3"], ITERS[2], FINAL_FP32, h3_traj)

            nc.sync.dma_start(h3_d[:, :], h3_traj[:, TC : TC + 1])
            nc.sync.dma_start(c3_d[:, :], c3_sb[:, TC - 1 : TC])

            for s0, s1 in _chunks(TC, SEC):
                pl = psum.tile([128, min(CHUNK, TC)], _F32, tag="ps")
                nc.tensor.matmul(
                    pl[0:1, 0 : s1 - s0],
                    wt["wlinT"][:, 0:1],
                    h3_traj[:, 1 + s0 : 1 + s1].bitcast(_F32),
                    start=True,
                    stop=True,
                )
                ot = olin.tile([1, SEC], _F32, tag="ol")
                nc.scalar.activation(
                    ot[0:1, 0 : s1 - s0],
                    pl[0:1, 0 : s1 - s0],
                    _IDENT,
                    bias=wt["blin"][0:1, 0:1],
                )
                nc.sync.dma_start(outs_d[:, s0:s1], ot[0:1, 0 : s1 - s0])
    _split_multi_waits(nc)
    return nc


_CACHED_NC = None
_CACHED_EXEC = None


def _get_nc():
    global _CACHED_NC
    if _CACHED_NC is None:
        _CACHED_NC = _build_program()
    return _CACHED_NC


def _get_exec():
    """Build the program once and wrap it in a persistent jitted shard_map
    callable (run_bass_kernel_spmd re-lowers the whole module on every call,
    which costs ~0.5s for this program)."""
    global _CACHED_EXEC
    if _CACHED_EXEC is not None:
        return _CACHED_EXEC
    import jax
    from jax.sharding import Mesh, PartitionSpec
    from jax.experimental.shard_map import shard_map
    from concourse.bass2jax import (
        _bass_exec_p,
        partition_id_tensor,
        install_neuronx_cc_hook,
    )

    install_neuronx_cc_hook()
    nc = _get_nc()
    partition_name = nc.partition_id_tensor.name if nc.partition_id_tensor else None
    in_names, out_names, out_avals, zero_outs = [], [], [], []
    for alloc in nc.m.functions[0].allocations:
        if not isinstance(alloc, mybir.MemoryLocationSet):
            continue
        name = alloc.memorylocations[0].name
        if alloc.kind == "ExternalInput":
            if name != partition_name:
                in_names.append(name)
        elif alloc.kind == "ExternalOutput":
            shape = tuple(alloc.tensor_shape)
            dtype = mybir.dt.np(alloc.dtype)
            out_names.append(name)
            out_avals.append(jax.core.ShapedArray(shape, dtype))
            zero_outs.append(np.zeros(shape, dtype))
    n_params = len(in_names)
    in_names_full = in_names + out_names + ([partition_name] if partition_name else [])

    def _body(*args):
        operands = list(args)
        if partition_name is not None:
            operands.append(partition_id_tensor())
        return tuple(
            _bass_exec_p.bind(
                *operands,
                out_avals=tuple(out_avals),
                in_names=tuple(in_names_full),
                out_names=tuple(out_names),
                lowering_input_output_aliases=(),
                sim_require_finite=True,
                sim_require_nnan=True,
                nc=nc,
            )
        )

    devices = jax.devices()[:N_CORES]
    mesh = Mesh(np.asarray(devices), ("core",))
    n_outs = len(out_avals)
    sharded = jax.jit(
        shard_map(
            _body,
            mesh=mesh,
            in_specs=(PartitionSpec("core"),) * (n_params + n_outs),
            out_specs=(PartitionSpec("core"),) * n_outs,
            check_rep=False,
        ),
        keep_unused=True,
    )

    def run(in_maps):
        per_core = [[np.asarray(m[nm]) for nm in in_names] for m in in_maps]
        concat_in = [
            np.concatenate([per_core[c][i] for c in range(N_CORES)], axis=0)
            for i in range(n_params)
        ]
        concat_zeros = [
            np.zeros((N_CORES * z.shape[0], *z.shape[1:]), z.dtype)
            for z in zero_outs
        ]
        out_arrs = sharded(*concat_in, *concat_zeros)
        return [
            {
                name: np.asarray(out_arrs[i]).reshape(
                    N_CORES, *out_avals[i].shape
                )[c]
                for i, name in enumerate(out_names)
            }
            for c in range(N_CORES)
        ]

    _CACHED_EXEC = run
    return run


def _make_in_maps(inputs):
    f32 = lambda a: np.ascontiguousarray(np.asarray(a), dtype=np.float32)
    x = f32(inputs["input"])            # [T, IN]
    common = {}
    for ell, h in (("1", H1), ("2", H2), ("3", H3)):
        common[f"wih{ell}T"] = f32(np.asarray(inputs[f"W_ih{ell}"]).T)
        common[f"whh{ell}T"] = f32(np.asarray(inputs[f"W_hh{ell}"]).T)
        b = f32(inputs[f"b_ih{ell}"]) + f32(inputs[f"b_hh{ell}"])
        common[f"b{ell}"] = f32(b.reshape(4, h).T)
    common["wlinT"] = f32(np.asarray(inputs["W_lin"]).T)
    common["blin"] = f32(np.asarray(inputs["b_lin"]).reshape(1, 1))
    in_maps = []
    for k in range(N_CORES):
        s0 = 0 if k == 0 else k * CH - WARM
        m = dict(common)
        m["xT"] = f32(x[s0 : s0 + TC].T)
        in_maps.append(m)
    return in_maps


def kernel(**inputs):
    run = _get_exec()
    in_maps = _make_in_maps(inputs)
    results = run(in_maps)
    outputs = np.empty((1, T), dtype=np.float32)
    for k in range(N_CORES):
        o = results[k]["outs"][0]
        if k == 0:
            outputs[0, 0:CH] = o[0:CH]
        else:
            outputs[0, k * CH : (k + 1) * CH] = o[WARM : WARM + CH]
    h3 = results[N_CORES - 1]["h3_last"].reshape(1, H3).astype(np.float32)
    c3 = results[N_CORES - 1]["c3_last"].reshape(1, H3)
    return outputs, h3, c3
